# revision 1
# baseline (speedup 1.0000x reference)
"""Trainium2 Bass kernel for nn_HKRPQParallelBlock (RPQ-quantized parallel
transformer block: LN -> in_proj (dequant GEMM) -> [MLP | SDPA] -> out_proj
(dequant GEMM) -> fold + residual).

Sharding (8 cores, zero device-to-device communication):
  - Each core computes a 896-row slice of in_proj (512 MLP rows + q/k/v rows of
    2 heads), cluster-aligned so the RPQ dequant gathers stay uniform.
  - Each core runs SDPA for its 2 heads over all 4 batches.
  - out_proj is sharded over its contraction dim (the 640 activation features
    this core produced); every core emits a full partial (1024, 4096) output
    and the host sums the 8 partials (+ residual).
  - The OUT_OUT->DIM fold (o[:, :1024] + o[:, 1024:]) is folded into the
    dequantized weight before the GEMM, halving out_proj FLOPs.

All GEMMs run in float32r (1 cycle/row on the PE at N>=512, ~1.4e-4 rel err).
Dequant uses InstDMAGatherAnt (one gather per (level, tile): 256B sub-vectors).
"""

import os
import numpy as np
import concourse.bass as bass
import concourse.bacc as bacc
import concourse.tile as tile
import concourse.mybir as mybir
from concourse.bass_utils import run_bass_kernel_spmd
from concourse.masks import make_identity
from contextlib import ExitStack

F32 = mybir.dt.float32
F32R = mybir.dt.float32r
I16 = mybir.dt.int16
AF = mybir.ActivationFunctionType
ALU = mybir.AluOpType

NCORES = 8
DIM = 1024
HEADS = 16
HD = 64
MLP = 4 * DIM                 # 4096
IN_OUT = MLP + 3 * DIM        # 7168
OUT_IN = MLP + DIM            # 5120
OUT_OUT = 2 * DIM             # 2048
R = 2
K = 64
NCB = 16
SUB_IN = 64
SUB_OUT = 320
IN_CLUSTERS = 112
OUT_CLUSTERS = 16
B, N = 4, 1024
TOK = B * N                   # 4096
EPS = 1e-5
SCALE = HD ** -0.5            # 0.125

F_TILES = 7                   # per-core in_proj feature tiles of 128 rows
MLP_PER_CORE = MLP // NCORES  # 512
HEADS_PER_CORE = 2
N_CBK = 4                     # padded per-core out_cb codebook count

_BUILD_CACHE = {}


def _row_base(core, t):
    """Global in_proj row of the first row of per-core feature tile t."""
    if t < 4:
        return MLP_PER_CORE * core + 128 * t
    return MLP + DIM * (t - 4) + 128 * core


def _chunk_map(core):
    """Per-core out_proj contraction columns as 10 chunks of 64.

    Returns (chunks, cbks): chunks = [(local_cbk_pos, sub_chunk)] * 10 in the
    order matching the rhs K-tiles (4 MLP hT tiles then xaT), cbks = the <=4
    distinct global codebook columns, padded to N_CBK.
    """
    cols = [MLP_PER_CORE * core + 64 * k for k in range(8)] + \
           [MLP + 128 * core + 64 * k for k in range(2)]
    raw = [(g // SUB_OUT, (g % SUB_OUT) // 64) for g in cols]
    cbks = sorted({cb for cb, _ in raw})
    assert len(cbks) <= N_CBK, cbks
    while len(cbks) < N_CBK:
        cbks.append(cbks[-1])
    pos = {cb: i for i, cb in enumerate(cbks)}
    chunks = [(pos[cb], s) for cb, s in raw]
    return chunks, cbks


def _wrap_idx(flat):
    """Logical index list -> wrapped [16, n/16] block replicated to 128 parts."""
    n = flat.shape[0]
    blk = np.zeros((16, n // 16), np.int16)
    blk[np.arange(n) % 16, np.arange(n) // 16] = flat.astype(np.int16)
    return np.tile(blk, (8, 1))


def _build_nc():
    if "nc" in _BUILD_CACHE:
        return _BUILD_CACHE["nc"]

    nc = bacc.Bacc("TRN2", target_bir_lowering=False, debug=False,
                   num_devices=NCORES)

    x_d = nc.dram_tensor("x4096", (TOK, DIM), F32, kind="ExternalInput")
    incb_d = nc.dram_tensor("in_cb_s", (R, F_TILES, 2, NCB, K, SUB_IN), F32,
                            kind="ExternalInput")
    outcb_d = nc.dram_tensor("out_cb_s", (R, OUT_CLUSTERS, N_CBK, K, SUB_OUT),
                             F32, kind="ExternalInput")
    winidx_d = nc.dram_tensor("win_idx", (R, F_TILES, 128, 128), I16,
                              kind="ExternalInput")
    woutidx_d = nc.dram_tensor("wout_idx", (R, OUT_CLUSTERS, 128, 80), I16,
                               kind="ExternalInput")
    o_d = nc.dram_tensor("o_t", (DIM, TOK), F32, kind="ExternalOutput")

    with ExitStack() as ctx, nc.allow_low_precision(reason="fp32r matmul feeds"):
        tc = ctx.enter_context(tile.TileContext(nc))
        const = ctx.enter_context(tc.tile_pool(name="const", bufs=1))
        wpool = ctx.enter_context(tc.tile_pool(name="wpool", bufs=1))
        stage = ctx.enter_context(tc.tile_pool(name="stage", bufs=2))
        stage4 = ctx.enter_context(tc.tile_pool(name="stage4", bufs=4))
        work = ctx.enter_context(tc.tile_pool(name="work", bufs=2))
        small = ctx.enter_context(tc.tile_pool(name="small", bufs=2))
        psA = ctx.enter_context(tc.tile_pool(name="psA", bufs=4, space="PSUM"))
        psT = ctx.enter_context(tc.tile_pool(name="psT", bufs=2, space="PSUM"))
        psV = ctx.enter_context(tc.tile_pool(name="psV", bufs=1, space="PSUM"))
        ptpool = ctx.enter_context(tc.tile_pool(name="ptpool", bufs=3))

        ident_f = const.tile([128, 128], F32, tag="ident_f")
        make_identity(nc, ident_f[:])
        ident_r = const.tile([128, 128], F32R, tag="ident_r")
        nc.vector.tensor_copy(ident_r[:], ident_f[:])
        ones_f = const.tile([128, 1], F32, tag="ones_f")
        nc.gpsimd.memset(ones_f[:], 1.0)
        ones_r = const.tile([1, 64], F32R, tag="ones_r")
        nc.vector.tensor_copy(ones_r[:], ones_f[:1, :].to_broadcast([1, 64]))
        ones_col_r = const.tile([128, 1], F32R, tag="ones_col_r")
        nc.vector.tensor_copy(ones_col_r[:], ones_f[:])

        winT = [wpool.tile([128, F_TILES * 128], F32R, name=f"winT{d}", tag=f"winT{d}")
                for d in range(8)]
        wfoldT = [wpool.tile([128, 1024], F32R, name=f"wfT{kk}", tag=f"wfT{kk}")
                  for kk in range(5)]

        # ---------------- emission helpers ------------------------------
        def emit_ln(b, xnT):
            """LayerNorm + PE transpose of batch b tokens into xnT tiles."""
            for tt in range(8):
                xt = work.tile([128, DIM], F32, name="xt", tag="xt")
                nc.sync.dma_start(
                    xt[:], x_d.ap()[b * N + tt * 128: b * N + (tt + 1) * 128, :])
                bstat = small.tile([128, 2, 6], F32, name="bstat", tag="bstat")
                nc.vector.bn_stats(bstat[:, 0, :], xt[:, :512])
                nc.vector.bn_stats(bstat[:, 1, :], xt[:, 512:])
                baggr = small.tile([128, 2], F32, name="baggr", tag="baggr")
                nc.vector.bn_aggr(baggr[:], bstat[:])
                veps = small.tile([128, 1], F32, name="veps", tag="veps")
                nc.vector.tensor_scalar_add(veps[:], baggr[:, 1:2], EPS)
                sd = small.tile([128, 1], F32, name="sd", tag="sd")
                nc.scalar.sqrt(sd[:], veps[:])
                rs = small.tile([128, 1], F32, name="rs", tag="rs")
                nc.vector.reciprocal(rs[:], sd[:])
                nmu = small.tile([128, 1], F32, name="nmu", tag="nmu")
                nc.vector.scalar_tensor_tensor(nmu[:], baggr[:, 0:1], -1.0,
                                               rs[:], op0=ALU.mult,
                                               op1=ALU.mult)
                xn = work.tile([128, DIM], F32R, name="xn", tag="xn")
                nc.scalar.activation(xn[:], xt[:], AF.Identity,
                                     bias=nmu[:], scale=rs[:])
                for d in range(8):
                    pst = psT.tile([128, 128], F32R, name="pstr", tag="pstr")
                    nc.tensor.matmul(pst[:], xn[:, d * 128:(d + 1) * 128],
                                     ident_r[:], is_transpose=True,
                                     start=True, stop=True)
                    nc.vector.tensor_copy(
                        xnT[d][:, tt * 128:(tt + 1) * 128], pst[:])

        def emit_w1():
            """in_proj dequant: gather both levels, transpose-accumulate."""
            for t in range(F_TILES):
                g = []
                for r in range(R):
                    gi = stage.tile([128, 128], I16, name=f"winidx{r}",
                                    tag=f"winidx{r}", bufs=7)
                    nc.sync.dma_start(gi[:], winidx_d.ap()[r, t])
                    gg = stage.tile([128, NCB, SUB_IN], F32,
                                    name=f"winstage{r}", tag=f"winstage{r}")
                    view = incb_d.ap()[r, t].rearrange("a c k s -> (a c k) s")
                    nc.gpsimd.dma_gather(gg[:], view, gi[:], 2048, 2048,
                                         SUB_IN, single_packet=False)
                    g.append(gg[:].rearrange("p c s -> p (c s)"))
                for d in range(8):
                    ps = psT.tile([128, 128], F32, name="pstr", tag="pstr")
                    nc.tensor.matmul(ps[:], g[0][:, d * 128:(d + 1) * 128],
                                     ident_f[:], is_transpose=True,
                                     start=True, stop=False)
                    nc.tensor.matmul(ps[:], g[1][:, d * 128:(d + 1) * 128],
                                     ident_f[:], is_transpose=True,
                                     start=False, stop=True)
                    nc.scalar.copy(winT[d][:, t * 128:(t + 1) * 128], ps[:])

        def emit_w2():
            """out_proj dequant: 4 gathers per cluster pair fold in PSUM."""
            for j in range(8):
                w4 = []
                for i, (r, jj) in enumerate([(0, j), (1, j),
                                             (0, j + 8), (1, j + 8)]):
                    gi = stage4.tile([128, 80], I16, name="woutidx",
                                     tag="woutidx", bufs=8)
                    nc.sync.dma_start(gi[:], woutidx_d.ap()[r, jj])
                    gg = stage4.tile([128, 10, 64], F32, name="woutstage",
                                     tag="woutstage")
                    view = outcb_d.ap()[r, jj].rearrange(
                        "c k (f s) -> (c k f) s", s=64)
                    nc.gpsimd.dma_gather(gg[:], view, gi[:], 1280, 1280, 64,
                                         single_packet=False)
                    w4.append(gg[:].rearrange("p c s -> p (c s)"))
                for kk in range(5):
                    ps = psT.tile([128, 128], F32, name="pstr", tag="pstr")
                    for i in range(4):
                        nc.tensor.matmul(ps[:],
                                         w4[i][:, kk * 128:(kk + 1) * 128],
                                         ident_f[:], is_transpose=True,
                                         start=(i == 0), stop=(i == 3))
                    nc.vector.tensor_copy(
                        wfoldT[kk][:, j * 128:(j + 1) * 128], ps[:])

        def emit_inproj(xnT, hT):
            for f in range(F_TILES):
                pss = [psA.tile([128, 512], F32, name="mm", tag="mm")
                       for _ in range(2)]
                for d in range(8):
                    for qc in range(2):
                        nc.tensor.matmul(
                            pss[qc][:], winT[d][:, f * 128:(f + 1) * 128],
                            xnT[d][:, qc * 512:(qc + 1) * 512],
                            start=(d == 0), stop=(d == 7))
                for qc in range(2):
                    nc.scalar.copy(hT[f][:, qc * 512:(qc + 1) * 512],
                                   pss[qc][:])

        def emit_attn(hT, xaT):
            vaugs = []
            for hh in range(HEADS_PER_CORE):
                lo, hi = hh * 64, (hh + 1) * 64
                vaug = wpool.tile([128, 8, 65], F32R, name=f"vaug{hh}",
                                  tag=f"vaug{hh}")
                for kvt in range(8):
                    psv = psT.tile([128, 128], F32R, name="pstr", tag="pstr")
                    nc.tensor.matmul(psv[:, :64],
                                     hT[6][lo:hi, kvt * 128:(kvt + 1) * 128],
                                     ident_r[lo:hi, lo:hi], is_transpose=True,
                                     start=True, stop=True)
                    nc.vector.tensor_copy(vaug[:, kvt, 0:64], psv[:, :64])
                    nc.vector.tensor_copy(vaug[:, kvt, 64:65], ones_col_r[:])
                vaugs.append(vaug)
            for qc in range(2):
                psavs = [psV.tile([65, 512], F32, name=f"av{hh}",
                                  tag=f"av{hh}") for hh in range(2)]
                prev = None
                for kvt in range(8):
                    cur = []
                    for hh in range(HEADS_PER_CORE):
                        lo, hi = hh * 64, (hh + 1) * 64
                        ps = psA.tile([128, 512], F32, name="mm", tag="mm")
                        nc.tensor.matmul(
                            ps[:], hT[5][lo:hi, kvt * 128:(kvt + 1) * 128],
                            hT[4][lo:hi, qc * 512:(qc + 1) * 512],
                            start=True, stop=True)
                        ptk = ptpool.tile([128, 512], F32R, name=f"PT{hh}",
                                          tag=f"PT{hh}")
                        nc.scalar.activation(ptk[:], ps[:], AF.Exp,
                                             scale=SCALE)
                        cur.append(ptk)
                    if prev is not None:
                        for hh in range(HEADS_PER_CORE):
                            nc.tensor.matmul(psavs[hh][:],
                                             vaugs[hh][:, kvt - 1, :],
                                             prev[hh][:],
                                             start=(kvt == 1), stop=False)
                    prev = cur
                for hh in range(HEADS_PER_CORE):
                    nc.tensor.matmul(psavs[hh][:], vaugs[hh][:, 7, :],
                                     prev[hh][:], start=False, stop=True)
                for hh in range(HEADS_PER_CORE):
                    lo, hi = hh * 64, (hh + 1) * 64
                    psav = psavs[hh]
                    rec = small.tile([1, 512], F32R, name="rec", tag="rec")
                    nc.vector.reciprocal(rec[:], psav[64:65, :])
                    psb = psA.tile([64, 512], F32, name="mm", tag="mm")
                    nc.tensor.matmul(psb[:], ones_r[:], rec[:],
                                     start=True, stop=True)
                    bc = small.tile([64, 512], F32, name="bc_sb", tag="bc_sb")
                    nc.vector.tensor_copy(bc[:], psb[:])
                    nc.vector.tensor_tensor(
                        xaT[lo:hi, qc * 512:(qc + 1) * 512],
                        psav[0:64, :], bc[:], op=ALU.mult)

        def emit_outproj(b, hT, xaT):
            for ot in range(8):
                pss = [psA.tile([128, 512], F32, name="mm", tag="mm")
                       for _ in range(2)]
                for kk in range(5):
                    rhs = hT[kk] if kk < 4 else xaT
                    for qc in range(2):
                        nc.tensor.matmul(
                            pss[qc][:], wfoldT[kk][:, ot * 128:(ot + 1) * 128],
                            rhs[:, qc * 512:(qc + 1) * 512],
                            start=(kk == 0), stop=(kk == 4))
                for qc in range(2):
                    osb = work.tile([128, 512], F32, name="osb", tag="osb")
                    nc.vector.tensor_copy(osb[:], pss[qc][:])
                    nc.sync.dma_start(
                        o_d.ap()[ot * 128:(ot + 1) * 128,
                                 b * N + qc * 512: b * N + (qc + 1) * 512],
                        osb[:])

        # ---------------- emission order --------------------------------
        def alloc_xnT():
            return [wpool.tile([128, N], F32R, name=f"xnT{d}", tag=f"xnT{d}")
                    for d in range(8)]

        # BASS_REPEAT>1 re-emits the body k times for wall-delta timing
        for _rep in range(int(os.environ.get("BASS_REPEAT", "1"))):
            xnT_b = alloc_xnT()
            emit_ln(0, xnT_b)
            emit_w1()
            for b in range(B):
                hT_b = [wpool.tile([128, N], F32R, name=f"hT{f}", tag=f"hT{f}")
                        for f in range(F_TILES)]
                emit_inproj(xnT_b, hT_b)
                if b == 0:
                    emit_w2()
                if b + 1 < B:
                    xnT_next = alloc_xnT()
                    emit_ln(b + 1, xnT_next)
                xaT_b = wpool.tile([128, N], F32R, name="xaT", tag="xaT")
                emit_attn(hT_b, xaT_b)
                emit_outproj(b, hT_b, xaT_b)
                if b + 1 < B:
                    xnT_b = xnT_next

    nc.compile()
    _BUILD_CACHE["nc"] = nc
    return nc


def make_in_maps(x, in_codebooks, in_indices, out_codebooks, out_indices):
    """Host-side sharding: slice codebooks, flatten gather indices per core."""
    x4096 = np.ascontiguousarray(x.reshape(TOK, DIM).astype(np.float32))
    in_maps = []
    p_arange = np.arange(128)
    cc_arange = np.arange(NCB)
    for c in range(NCORES):
        # in_proj codebook slice: 7 cluster pairs
        cl0 = [_row_base(c, t) // 64 for t in range(F_TILES)]
        incb = np.stack([in_codebooks[:, cl: cl + 2] for cl in cl0], axis=1)
        # (R, 7, 2, NCB, K, SUB_IN)
        win_idx = np.zeros((R, F_TILES, 128, 128), np.int16)
        for r in range(R):
            for t in range(F_TILES):
                rb = _row_base(c, t)
                iv = in_indices[r, rb: rb + 128, :]          # (128, NCB)
                # logical j = cc*128 + p ; local row in [2048,64] view
                flat = ((p_arange[None, :] // 64) * (NCB * K)
                        + cc_arange[:, None] * K + iv.T).reshape(-1)
                win_idx[r, t] = _wrap_idx(flat)

        # out_proj codebook slice + fold indices
        chunks, cbks = _chunk_map(c)
        outcb = np.ascontiguousarray(out_codebooks[:, :, cbks])
        # (R, OUT_CLUSTERS, N_CBK, K, SUB_OUT)
        wout_idx = np.zeros((R, OUT_CLUSTERS, 128, 80), np.int16)
        cbl = np.array([p for p, _ in chunks])               # (10,)
        sub = np.array([s for _, s in chunks])               # (10,)
        gcbk = np.array(cbks)
        for r in range(R):
            for j in range(OUT_CLUSTERS):
                ov = out_indices[r, j * 128:(j + 1) * 128, :]  # (128, NCB)
                idxv = ov[:, gcbk[cbl]]                        # (128, 10)
                flat = ((cbl[None, :] * K + idxv) * 5
                        + sub[None, :]).T.reshape(-1)          # ch-major
                wout_idx[r, j] = _wrap_idx(flat)

        in_maps.append({
            "x4096": x4096,
            "in_cb_s": np.ascontiguousarray(incb.astype(np.float32)),
            "out_cb_s": np.ascontiguousarray(outcb.astype(np.float32)),
            "win_idx": win_idx,
            "wout_idx": wout_idx,
        })
    return in_maps


def combine_outputs(x, results):
    o_sum = np.zeros((DIM, TOK), np.float64)
    for rmap in results:
        o_sum += rmap["o_t"].astype(np.float64)
    out = x.reshape(TOK, DIM).astype(np.float64) + o_sum.T
    return out.reshape(B, N, DIM).astype(np.float32)


def kernel(x, in_codebooks, in_indices, out_codebooks, out_indices):
    nc = _build_nc()
    in_maps = make_in_maps(x, in_codebooks, in_indices,
                           out_codebooks, out_indices)
    res = run_bass_kernel_spmd(nc, in_maps, core_ids=list(range(NCORES)))
    return combine_outputs(x, [res.results[c] for c in range(NCORES)])



# revision 7
# speedup vs baseline: 1.8223x; 1.8223x over previous
"""Trainium2 Bass kernel for nn_HKRPQParallelBlock (RPQ-quantized parallel
transformer block: LN -> in_proj (dequant GEMM) -> [MLP | SDPA] -> out_proj
(dequant GEMM) -> fold + residual).

Sharding (8 cores, zero device-to-device communication):
  - Each core computes a 896-row slice of in_proj (512 MLP rows + q/k/v rows of
    2 heads), cluster-aligned so the RPQ structure stays uniform.
  - Each core runs SDPA for its 2 heads over all 4 batches.
  - out_proj is sharded over its contraction dim (the 640 activation features
    this core produced); every core emits a full partial (1024, 4096) output
    and the host sums the 8 partials (+ residual).
  - The OUT_OUT->DIM fold (o[:, :1024] + o[:, 1024:]) is folded into the
    dequantized weight before the GEMM, halving out_proj FLOPs.

v2 vs v1:
  - RPQ dequant runs as one-hot x codebook matmuls on the PE (bf16) instead
    of SWDGE DMA gathers: the host ships pre-built one-hot index matrices and
    block-diagonal codebook tiles; the PE contracts them straight into the
    transposed weight layout the GEMMs need. No gather descriptors, no Pool
    engine DGE work, and the dequant output needs no separate transpose pass.
  - All matmul operands are bf16 (PSUM stays f32), halving SBUF footprint,
    DMA bytes and vector/scalar element work. PE transposes run 1 cycle/row.
  - DMAs are fewer and bigger, split across the sync (x in / o out) and
    vector (weights) queues; PSUM->SBUF copies are spread over the scalar,
    vector and pool engines.
"""

import os
import numpy as np
import concourse.bass as bass
import concourse.bacc as bacc
import concourse.tile as tile
import concourse.mybir as mybir
from concourse.bass_utils import run_bass_kernel_spmd
from concourse.masks import make_identity
from contextlib import ExitStack

F32 = mybir.dt.float32
F32R = mybir.dt.float32r
BF16 = mybir.dt.bfloat16
AF = mybir.ActivationFunctionType
ALU = mybir.AluOpType

NCORES = 8
DIM = 1024
HEADS = 16
HD = 64
MLP = 4 * DIM                 # 4096
IN_OUT = MLP + 3 * DIM        # 7168
OUT_IN = MLP + DIM            # 5120
OUT_OUT = 2 * DIM             # 2048
R = 2
K = 64
NCB = 16
SUB_IN = 64
SUB_OUT = 320
IN_CLUSTERS = 112
OUT_CLUSTERS = 16
B, N = 4, 1024
TOK = B * N                   # 4096
EPS = 1e-5
SCALE = HD ** -0.5            # 0.125

F_TILES = 7                   # per-core in_proj feature tiles of 128 rows
MLP_PER_CORE = MLP // NCORES  # 512
HEADS_PER_CORE = 2
KK = 5                        # out_proj contraction tiles of 128 per core

NPBF16 = mybir.dt.np(BF16)

_BUILD_CACHE = {}


def _row_base(core, t):
    """Global in_proj row of the first row of per-core feature tile t."""
    if t < 4:
        return MLP_PER_CORE * core + 128 * t
    return MLP + DIM * (t - 4) + 128 * core


def _chunk_cols(core):
    """The 10 global out_proj contraction columns (as 64-wide chunks) this
    core owns, in rhs order: 8 MLP chunks then 2 attention chunks."""
    return [MLP_PER_CORE * core + 64 * k for k in range(8)] + \
           [MLP + 128 * core + 64 * k for k in range(2)]


def _build_nc():
    if "nc" in _BUILD_CACHE:
        return _BUILD_CACHE["nc"]

    nc = bacc.Bacc("TRN2", target_bir_lowering=False, debug=False,
                   num_devices=NCORES)

    x_d = nc.dram_tensor("x4096", (TOK, DIM), F32, kind="ExternalInput")
    icb_d = nc.dram_tensor("icb", (F_TILES, 128, R, 2, 8, 128), BF16,
                           kind="ExternalInput")
    ioh_d = nc.dram_tensor("ioh", (F_TILES, 128, R, 2, 8, 64), BF16,
                           kind="ExternalInput")
    ocb_d = nc.dram_tensor("ocb", (8, 128, R, 2, KK, 128), BF16,
                           kind="ExternalInput")
    ooh_d = nc.dram_tensor("ooh", (8, 128, R, 2, KK, 128), BF16,
                           kind="ExternalInput")
    o_d = nc.dram_tensor("o_t", (DIM, TOK), F32, kind="ExternalOutput")

    with ExitStack() as ctx, nc.allow_low_precision(reason="bf16 matmul feeds"):
        tc = ctx.enter_context(tile.TileContext(nc))
        const = ctx.enter_context(tc.tile_pool(name="const", bufs=1))
        wpool = ctx.enter_context(tc.tile_pool(name="wpool", bufs=1))
        stage = ctx.enter_context(tc.tile_pool(name="stage", bufs=2))
        work = ctx.enter_context(tc.tile_pool(name="work", bufs=2))
        small = ctx.enter_context(tc.tile_pool(name="small", bufs=2))
        psA = ctx.enter_context(tc.tile_pool(name="psA", bufs=4, space="PSUM"))
        psT = ctx.enter_context(tc.tile_pool(name="psT", bufs=2, space="PSUM"))
        psV = ctx.enter_context(tc.tile_pool(name="psV", bufs=1, space="PSUM"))
        ptpool = ctx.enter_context(tc.tile_pool(name="ptpool", bufs=3))

        ident_f = const.tile([128, 128], F32, tag="ident_f")
        make_identity(nc, ident_f[:])
        ident_b = const.tile([128, 128], BF16, tag="ident_b")
        nc.vector.tensor_copy(ident_b[:], ident_f[:])
        ones_f = const.tile([128, 1], F32, tag="ones_f")
        nc.gpsimd.memset(ones_f[:], 1.0)
        ones_r = const.tile([1, 64], F32R, tag="ones_r")
        nc.vector.tensor_copy(ones_r[:], ones_f[:1, :].to_broadcast([1, 64]))

        winT = [wpool.tile([128, F_TILES * 128], BF16, name=f"winT{d}",
                           tag=f"winT{d}") for d in range(8)]
        wfoldT = [wpool.tile([128, 1024], BF16, name=f"wfT{kk}",
                             tag=f"wfT{kk}") for kk in range(KK)]

        # ---------------- emission helpers ------------------------------
        def emit_ln(b, xnT):
            """LayerNorm + PE transpose of batch b tokens into xnT tiles."""
            for tt in range(8):
                xt = stage.tile([128, DIM], F32, name="xt", tag="xt")
                nc.sync.dma_start(
                    xt[:], x_d.ap()[b * N + tt * 128: b * N + (tt + 1) * 128, :])
                bstat = small.tile([128, 2, 6], F32, name="bstat", tag="bstat")
                nc.vector.bn_stats(bstat[:, 0, :], xt[:, :512])
                nc.vector.bn_stats(bstat[:, 1, :], xt[:, 512:])
                baggr = small.tile([128, 2], F32, name="baggr", tag="baggr")
                nc.vector.bn_aggr(baggr[:], bstat[:])
                veps = small.tile([128, 1], F32, name="veps", tag="veps")
                nc.vector.tensor_scalar_add(veps[:], baggr[:, 1:2], EPS)
                sd = small.tile([128, 1], F32, name="sd", tag="sd")
                nc.scalar.sqrt(sd[:], veps[:])
                rs = small.tile([128, 1], F32, name="rs", tag="rs")
                nc.vector.reciprocal(rs[:], sd[:])
                nmu = small.tile([128, 1], F32, name="nmu", tag="nmu")
                nc.vector.scalar_tensor_tensor(nmu[:], baggr[:, 0:1], -1.0,
                                               rs[:], op0=ALU.mult,
                                               op1=ALU.mult)
                xn = stage.tile([128, DIM], BF16, name="xn", tag="xn")
                nc.scalar.activation(xn[:], xt[:], AF.Identity,
                                     bias=nmu[:], scale=rs[:])
                for d in range(8):
                    pst = psT.tile([128, 128], BF16, name="pstb", tag="pstr")
                    nc.tensor.matmul(pst[:], xn[:, d * 128:(d + 1) * 128],
                                     ident_b[:], is_transpose=True,
                                     start=True, stop=True)
                    if d % 2:
                        nc.scalar.copy(
                            xnT[d][:, tt * 128:(tt + 1) * 128], pst[:])
                    else:
                        nc.vector.tensor_copy(
                            xnT[d][:, tt * 128:(tt + 1) * 128], pst[:])

        def emit_w1():
            """in_proj dequant: one-hot x codebook matmuls, 2 levels fused."""
            for t in range(F_TILES):
                cbt = stage.tile([128, R, 2, 8, 128], BF16, name="icbt",
                                 tag="icbt")
                nc.gpsimd.dma_start(
                    cbt[:].rearrange("p r h d f -> p (r h d f)"),
                    icb_d.ap()[t].rearrange("p r h d f -> p (r h d f)"))
                oht = stage.tile([128, R, 2, 8, 64], BF16, name="ioht",
                                 tag="ioht")
                nc.gpsimd.dma_start(
                    oht[:].rearrange("p r h d f -> p (r h d f)"),
                    ioh_d.ap()[t].rearrange("p r h d f -> p (r h d f)"))
                for d in range(8):
                    ps = psT.tile([128, 128], F32, name="pstr", tag="pstr")
                    for h in range(2):
                        for r in range(R):
                            nc.tensor.matmul(ps[:, h * 64:(h + 1) * 64],
                                             cbt[:, r, h, d, :],
                                             oht[:, r, h, d, :],
                                             start=(r == 0), stop=(r == 1))
                    if d % 2:
                        nc.scalar.copy(winT[d][:, t * 128:(t + 1) * 128], ps[:])
                    else:
                        nc.vector.tensor_copy(
                            winT[d][:, t * 128:(t + 1) * 128], ps[:])

        def emit_w2():
            """out_proj dequant: one-hot matmuls, levels+fold accumulated."""
            for ot in range(8):
                cbt = stage.tile([128, R, 2, KK, 128], BF16, name="ocbt",
                                 tag="ocbt")
                nc.gpsimd.dma_start(
                    cbt[:].rearrange("p r f k c -> p (r f k c)"),
                    ocb_d.ap()[ot].rearrange("p r f k c -> p (r f k c)"))
                oht = stage.tile([128, R, 2, KK, 128], BF16, name="ooht",
                                 tag="ooht")
                nc.gpsimd.dma_start(
                    oht[:].rearrange("p r f k c -> p (r f k c)"),
                    ooh_d.ap()[ot].rearrange("p r f k c -> p (r f k c)"))
                for kk in range(KK):
                    ps = psT.tile([128, 128], F32, name="pstr", tag="pstr")
                    i = 0
                    for r in range(R):
                        for fh in range(2):
                            nc.tensor.matmul(ps[:], cbt[:, r, fh, kk, :],
                                             oht[:, r, fh, kk, :],
                                             start=(i == 0), stop=(i == 3))
                            i += 1
                    if kk % 2:
                        nc.scalar.copy(wfoldT[kk][:, ot * 128:(ot + 1) * 128],
                                       ps[:])
                    else:
                        nc.vector.tensor_copy(
                            wfoldT[kk][:, ot * 128:(ot + 1) * 128], ps[:])

        def emit_inproj(xnT, hT):
            for f in range(F_TILES):
                pss = [psA.tile([128, 512], F32, name="mm", tag="mm")
                       for _ in range(2)]
                for d in range(8):
                    for qc in range(2):
                        nc.tensor.matmul(
                            pss[qc][:], winT[d][:, f * 128:(f + 1) * 128],
                            xnT[d][:, qc * 512:(qc + 1) * 512],
                            start=(d == 0), stop=(d == 7))
                nc.scalar.copy(hT[f][:, 0:512], pss[0][:])
                nc.vector.tensor_copy(hT[f][:, 512:1024], pss[1][:])

        def emit_attn(hT, xaT):
            vaugs = []
            for hh in range(HEADS_PER_CORE):
                lo, hi = hh * 64, (hh + 1) * 64
                vaug = wpool.tile([128, 8, 65], BF16, name=f"vaug{hh}",
                                  tag=f"vaug{hh}")
                for kvt in range(8):
                    psv = psT.tile([128, 128], BF16, name="pstb", tag="pstr")
                    nc.tensor.matmul(psv[:, :64],
                                     hT[6][lo:hi, kvt * 128:(kvt + 1) * 128],
                                     ident_b[lo:hi, lo:hi], is_transpose=True,
                                     start=True, stop=True)
                    nc.vector.tensor_copy(vaug[:, kvt, 0:64], psv[:, :64])
                nc.vector.memset(vaug[:, :, 64:65], 1.0)
                vaugs.append(vaug)
            for qc in range(2):
                psavs = [psV.tile([65, 512], F32, name=f"av{hh}",
                                  tag=f"av{hh}") for hh in range(2)]
                prev = None
                for kvt in range(8):
                    cur = []
                    for hh in range(HEADS_PER_CORE):
                        lo, hi = hh * 64, (hh + 1) * 64
                        ps = psA.tile([128, 512], F32, name="mm", tag="mm")
                        nc.tensor.matmul(
                            ps[:], hT[5][lo:hi, kvt * 128:(kvt + 1) * 128],
                            hT[4][lo:hi, qc * 512:(qc + 1) * 512],
                            start=True, stop=True)
                        ptk = ptpool.tile([128, 512], BF16, name=f"PT{hh}",
                                          tag=f"PT{hh}")
                        nc.scalar.activation(ptk[:], ps[:], AF.Exp,
                                             scale=SCALE)
                        cur.append(ptk)
                    if prev is not None:
                        for hh in range(HEADS_PER_CORE):
                            nc.tensor.matmul(psavs[hh][:],
                                             vaugs[hh][:, kvt - 1, :],
                                             prev[hh][:],
                                             start=(kvt == 1), stop=False)
                    prev = cur
                for hh in range(HEADS_PER_CORE):
                    nc.tensor.matmul(psavs[hh][:], vaugs[hh][:, 7, :],
                                     prev[hh][:], start=False, stop=True)
                for hh in range(HEADS_PER_CORE):
                    lo, hi = hh * 64, (hh + 1) * 64
                    psav = psavs[hh]
                    rec = small.tile([1, 512], F32R, name="rec", tag="rec")
                    nc.vector.reciprocal(rec[:], psav[64:65, :])
                    psb = psA.tile([64, 512], F32, name="mm", tag="mm")
                    nc.tensor.matmul(psb[:], ones_r[:], rec[:],
                                     start=True, stop=True)
                    bc = small.tile([64, 512], F32, name="bc_sb", tag="bc_sb")
                    nc.vector.tensor_copy(bc[:], psb[:])
                    nc.vector.tensor_tensor(
                        xaT[lo:hi, qc * 512:(qc + 1) * 512],
                        psav[0:64, :], bc[:], op=ALU.mult)

        def emit_outproj(b, hT, xaT):
            for ot in range(8):
                pss = [psA.tile([128, 512], F32, name="mm", tag="mm")
                       for _ in range(2)]
                for kk in range(KK):
                    rhs = hT[kk] if kk < 4 else xaT
                    for qc in range(2):
                        nc.tensor.matmul(
                            pss[qc][:], wfoldT[kk][:, ot * 128:(ot + 1) * 128],
                            rhs[:, qc * 512:(qc + 1) * 512],
                            start=(kk == 0), stop=(kk == KK - 1))
                osb = work.tile([128, 1024], F32, name="osb", tag="osb")
                nc.scalar.copy(osb[:, 0:512], pss[0][:])
                nc.vector.tensor_copy(osb[:, 512:1024], pss[1][:])
                nc.sync.dma_start(
                    o_d.ap()[ot * 128:(ot + 1) * 128, b * N: (b + 1) * N],
                    osb[:])

        # ---------------- emission order --------------------------------
        def alloc_xnT(par):
            return [wpool.tile([128, N], BF16, name=f"xnT{d}",
                               tag=f"xnT{d}p{par}") for d in range(8)]

        # BASS_REPEAT>1 re-emits the body k times for wall-delta timing
        for _rep in range(int(os.environ.get("BASS_REPEAT", "1"))):
            xnT_b = alloc_xnT(0)
            emit_ln(0, xnT_b)
            emit_w1()
            for b in range(B):
                par = b % 2
                hT_b = [wpool.tile([128, N], BF16, name=f"hT{f}",
                                   tag=f"hT{f}p{par}") for f in range(F_TILES)]
                emit_inproj(xnT_b, hT_b)
                if b == 0:
                    emit_w2()
                if b + 1 < B:
                    xnT_next = alloc_xnT(1 - par)
                    emit_ln(b + 1, xnT_next)
                xaT_b = wpool.tile([128, N], BF16, name="xaT", tag=f"xaTp{par}")
                emit_attn(hT_b, xaT_b)
                emit_outproj(b, hT_b, xaT_b)
                if b + 1 < B:
                    xnT_b = xnT_next

    nc.compile()
    _BUILD_CACHE["nc"] = nc
    return nc


def make_in_maps(x, in_codebooks, in_indices, out_codebooks, out_indices):
    """Host-side input marshalling: per-core one-hot index matrices and
    block-diagonal codebook tiles (bf16), plus the flattened activations.

    Pure layout/encoding transforms — all arithmetic (dequant sums, GEMMs,
    LN, SDPA) runs on device.
    """
    x4096 = np.ascontiguousarray(np.asarray(x).reshape(TOK, DIM)
                                 .astype(np.float32))
    in_cb = np.asarray(in_codebooks, np.float32)
    in_idx = np.asarray(in_indices)
    out_cb = np.asarray(out_codebooks, np.float32)
    out_idx = np.asarray(out_indices)
    eye = np.arange(K)

    in_maps = []
    for c in range(NCORES):
        # ---- in_proj: icb (7,128,R,2,8,128), ioh (7,128,R,2,8,64) ----
        rows = np.stack([np.arange(_row_base(c, t), _row_base(c, t) + 128)
                         for t in range(F_TILES)])            # (7,128)
        cl0 = np.array([_row_base(c, t) // 64 for t in range(F_TILES)])

        ivc = in_idx[:, rows, :]                              # (R,7,128,16)
        oh = (ivc[..., None] == eye).astype(NPBF16)           # (R,7,128,16,64)
        # axes: (r, t, (h,m), (d,ci), k) -> (t, ci, k, r, h, d, m)
        oh = oh.reshape(R, F_TILES, 2, 64, 8, 2, K)
        ioh = np.ascontiguousarray(
            oh.transpose(1, 5, 6, 0, 2, 4, 3)                 # t,ci,k,r,h,d,m
            .reshape(F_TILES, 128, R, 2, 8, 64))

        # cb tiles: (r, t, h, d, ci, k, s) from clusters cl0[t]+h
        cl_ids = cl0[:, None] + np.array([0, 1])              # (7,2)
        cbs = in_cb[:, cl_ids]                                # (R,7,2,16,64,64)
        cbs = cbs.reshape(R, F_TILES, 2, 8, 2, K, SUB_IN)
        icb = np.zeros((F_TILES, 128, R, 2, 8, 128), np.float32)
        for ci in range(2):
            # (r,t,h,d,k,s) -> (t,k,r,h,d,s)
            blk = cbs[:, :, :, :, ci].transpose(1, 4, 0, 2, 3, 5)
            icb[:, ci * 64:(ci + 1) * 64, :, :, :,
                ci * 64:(ci + 1) * 64] = blk
        icb = icb.astype(NPBF16)

        # ---- out_proj: ocb/ooh (8,128,R,2,5,128) -------------------------
        cols = _chunk_cols(c)
        gcbk = np.array([g // SUB_OUT for g in cols])          # (10,)
        gsub = np.array([(g % SUB_OUT) // 64 for g in cols])   # (10,)

        # per-chunk codebook slices: (R, 16 clusters, 10, K, 64)
        sel = np.empty((R, OUT_CLUSTERS, 10, K, 64), np.float32)
        for i in range(10):
            sel[:, :, i] = out_cb[:, :, gcbk[i], :,
                                  64 * gsub[i]: 64 * gsub[i] + 64]
        # (r, (fh,ot) cluster, (kk,ci), k, s) -> (ot, ci, k, r, fh, kk, s)
        sel = sel.reshape(R, 2, 8, KK, 2, K, 64)
        ocb = np.zeros((8, 128, R, 2, KK, 128), np.float32)
        for ci in range(2):
            # (r,fh,ot,kk,k,s) -> (ot,k,r,fh,kk,s)
            blk = sel[:, :, :, :, ci].transpose(2, 4, 0, 1, 3, 5)
            ocb[:, ci * 64:(ci + 1) * 64, :, :, :,
                ci * 64:(ci + 1) * 64] = blk
        ocb = ocb.astype(NPBF16)

        ov = out_idx[:, :, gcbk]                               # (R,2048,10)
        ooh_raw = (ov[..., None] == eye).astype(NPBF16)        # (R,2048,10,64)
        # rows: (r, (fh,ot,m), (kk,ci), k) -> (ot, ci, k, r, fh, kk, m)
        ooh_raw = ooh_raw.reshape(R, 2, 8, 128, KK, 2, K)
        ooh = np.ascontiguousarray(
            ooh_raw.transpose(2, 5, 6, 0, 1, 4, 3)
            .reshape(8, 128, R, 2, KK, 128))

        in_maps.append({
            "x4096": x4096,
            "icb": np.ascontiguousarray(icb),
            "ioh": ioh,
            "ocb": np.ascontiguousarray(ocb),
            "ooh": ooh,
        })
    return in_maps


def combine_outputs(x, results):
    o_sum = np.zeros((DIM, TOK), np.float64)
    for rmap in results:
        o_sum += rmap["o_t"].astype(np.float64)
    out = np.asarray(x).reshape(TOK, DIM).astype(np.float64) + o_sum.T
    return out.reshape(B, N, DIM).astype(np.float32)


def kernel(x, in_codebooks, in_indices, out_codebooks, out_indices):
    nc = _build_nc()
    in_maps = make_in_maps(x, in_codebooks, in_indices,
                           out_codebooks, out_indices)
    res = run_bass_kernel_spmd(nc, in_maps, core_ids=list(range(NCORES)))
    return combine_outputs(x, [res.results[c] for c in range(NCORES)])


# revision 8
# speedup vs baseline: 2.0445x; 1.1219x over previous
"""Trainium2 Bass kernel for nn_HKRPQParallelBlock (RPQ-quantized parallel
transformer block: LN -> in_proj (dequant GEMM) -> [MLP | SDPA] -> out_proj
(dequant GEMM) -> fold + residual).

Sharding (8 cores, zero device-to-device communication):
  - Each core computes a 896-row slice of in_proj (512 MLP rows + q/k/v rows of
    2 heads), cluster-aligned so the RPQ structure stays uniform.
  - Each core runs SDPA for its 2 heads over all 4 batches.
  - out_proj is sharded over its contraction dim (the 640 activation features
    this core produced); every core emits a full partial (1024, 4096) output
    and the host sums the 8 partials (+ residual).
  - The OUT_OUT->DIM fold (o[:, :1024] + o[:, 1024:]) is folded into the
    dequantized weight before the GEMM, halving out_proj FLOPs.

v2 vs v1:
  - RPQ dequant runs as one-hot x codebook matmuls on the PE (bf16) instead
    of SWDGE DMA gathers: the host ships pre-built one-hot index matrices and
    block-diagonal codebook tiles; the PE contracts them straight into the
    transposed weight layout the GEMMs need. No gather descriptors, no Pool
    engine DGE work, and the dequant output needs no separate transpose pass.
  - All matmul operands are bf16 (PSUM stays f32), halving SBUF footprint,
    DMA bytes and vector/scalar element work. PE transposes run 1 cycle/row.
  - DMAs are fewer and bigger, split across the sync (x in / o out) and
    vector (weights) queues; PSUM->SBUF copies are spread over the scalar,
    vector and pool engines.
"""

import os
import numpy as np
import concourse.bass as bass
import concourse.bacc as bacc
import concourse.tile as tile
import concourse.mybir as mybir
from concourse.bass_utils import run_bass_kernel_spmd
from concourse.masks import make_identity
from contextlib import ExitStack

F32 = mybir.dt.float32
F32R = mybir.dt.float32r
BF16 = mybir.dt.bfloat16
AF = mybir.ActivationFunctionType
ALU = mybir.AluOpType

NCORES = 8
DIM = 1024
HEADS = 16
HD = 64
MLP = 4 * DIM                 # 4096
IN_OUT = MLP + 3 * DIM        # 7168
OUT_IN = MLP + DIM            # 5120
OUT_OUT = 2 * DIM             # 2048
R = 2
K = 64
NCB = 16
SUB_IN = 64
SUB_OUT = 320
IN_CLUSTERS = 112
OUT_CLUSTERS = 16
B, N = 4, 1024
TOK = B * N                   # 4096
EPS = 1e-5
SCALE = HD ** -0.5            # 0.125

F_TILES = 7                   # per-core in_proj feature tiles of 128 rows
MLP_PER_CORE = MLP // NCORES  # 512
HEADS_PER_CORE = 2
KK = 5                        # out_proj contraction tiles of 128 per core

NPBF16 = mybir.dt.np(BF16)

# packed weight-blob section sizes (bf16 elements)
ICB_T = 128 * R * 2 * 8 * 128          # per f-tile: 524288
IOH_T = 128 * R * 2 * 8 * 64           # per f-tile: 262144
OCB_T = 128 * R * 2 * KK * 128         # per out-tile: 327680
IOH_OFF = F_TILES * ICB_T              # 3670016
OCB_OFF = IOH_OFF + F_TILES * IOH_T    # 5505024
OOH_OFF = OCB_OFF + 8 * OCB_T          # 8126464
WB_TOTAL = OOH_OFF + 8 * OCB_T         # 10747904

_BUILD_CACHE = {}


def _row_base(core, t):
    """Global in_proj row of the first row of per-core feature tile t."""
    if t < 4:
        return MLP_PER_CORE * core + 128 * t
    return MLP + DIM * (t - 4) + 128 * core


def _chunk_cols(core):
    """The 10 global out_proj contraction columns (as 64-wide chunks) this
    core owns, in rhs order: 8 MLP chunks then 2 attention chunks."""
    return [MLP_PER_CORE * core + 64 * k for k in range(8)] + \
           [MLP + 128 * core + 64 * k for k in range(2)]


def _build_nc():
    if "nc" in _BUILD_CACHE:
        return _BUILD_CACHE["nc"]

    nc = bacc.Bacc("TRN2", target_bir_lowering=False, debug=False,
                   num_devices=NCORES)

    x_d = nc.dram_tensor("x4096", (TOK, DIM), F32, kind="ExternalInput")
    # One packed bf16 blob for all dequant operands (fewer per-call buffer
    # args + fewer, bigger DMAs): [icb | ioh | ocb | ooh], each section
    # t/ot-major with a contiguous [128, free] tile per slice.
    wblob_d = nc.dram_tensor("wblob", (WB_TOTAL,), BF16, kind="ExternalInput")
    o_d = nc.dram_tensor("o_t", (DIM, TOK), F32, kind="ExternalOutput")

    with ExitStack() as ctx, nc.allow_low_precision(reason="bf16 matmul feeds"):
        tc = ctx.enter_context(tile.TileContext(nc))
        const = ctx.enter_context(tc.tile_pool(name="const", bufs=1))
        wpool = ctx.enter_context(tc.tile_pool(name="wpool", bufs=1))
        stage = ctx.enter_context(tc.tile_pool(name="stage", bufs=2))
        work = ctx.enter_context(tc.tile_pool(name="work", bufs=2))
        small = ctx.enter_context(tc.tile_pool(name="small", bufs=2))
        psA = ctx.enter_context(tc.tile_pool(name="psA", bufs=4, space="PSUM"))
        psT = ctx.enter_context(tc.tile_pool(name="psT", bufs=2, space="PSUM"))
        psV = ctx.enter_context(tc.tile_pool(name="psV", bufs=1, space="PSUM"))
        ptpool = ctx.enter_context(tc.tile_pool(name="ptpool", bufs=3))

        ident_f = const.tile([128, 128], F32, tag="ident_f")
        make_identity(nc, ident_f[:])
        ident_b = const.tile([128, 128], BF16, tag="ident_b")
        nc.vector.tensor_copy(ident_b[:], ident_f[:])
        ones_f = const.tile([128, 1], F32, tag="ones_f")
        nc.gpsimd.memset(ones_f[:], 1.0)
        ones_r = const.tile([1, 64], F32R, tag="ones_r")
        nc.vector.tensor_copy(ones_r[:], ones_f[:1, :].to_broadcast([1, 64]))

        winT = [wpool.tile([128, F_TILES * 128], BF16, name=f"winT{d}",
                           tag=f"winT{d}") for d in range(8)]
        wfoldT = [wpool.tile([128, 1024], BF16, name=f"wfT{kk}",
                             tag=f"wfT{kk}") for kk in range(KK)]

        # ---------------- emission helpers ------------------------------
        def emit_ln(b, xnT):
            """LayerNorm + PE transpose of batch b tokens into xnT tiles."""
            for tt in range(8):
                xt = stage.tile([128, DIM], F32, name="xt", tag="xt")
                nc.sync.dma_start(
                    xt[:], x_d.ap()[b * N + tt * 128: b * N + (tt + 1) * 128, :])
                bstat = small.tile([128, 2, 6], F32, name="bstat", tag="bstat")
                nc.vector.bn_stats(bstat[:, 0, :], xt[:, :512])
                nc.vector.bn_stats(bstat[:, 1, :], xt[:, 512:])
                baggr = small.tile([128, 2], F32, name="baggr", tag="baggr")
                nc.vector.bn_aggr(baggr[:], bstat[:])
                veps = small.tile([128, 1], F32, name="veps", tag="veps")
                nc.vector.tensor_scalar_add(veps[:], baggr[:, 1:2], EPS)
                sd = small.tile([128, 1], F32, name="sd", tag="sd")
                nc.scalar.sqrt(sd[:], veps[:])
                rs = small.tile([128, 1], F32, name="rs", tag="rs")
                nc.vector.reciprocal(rs[:], sd[:])
                xn = stage.tile([128, DIM], BF16, name="xn", tag="xn")
                nc.vector.tensor_scalar(xn[:], xt[:], baggr[:, 0:1], rs[:],
                                        op0=ALU.subtract, op1=ALU.mult)
                for d in range(8):
                    pst = psT.tile([128, 128], BF16, name="pstb", tag="pstr")
                    nc.tensor.matmul(pst[:], xn[:, d * 128:(d + 1) * 128],
                                     ident_b[:], is_transpose=True,
                                     start=True, stop=True)
                    if d % 2:
                        nc.scalar.copy(
                            xnT[d][:, tt * 128:(tt + 1) * 128], pst[:])
                    else:
                        nc.vector.tensor_copy(
                            xnT[d][:, tt * 128:(tt + 1) * 128], pst[:])

        def emit_w1():
            """in_proj dequant: one-hot x codebook matmuls, 2 levels fused."""
            for t in range(F_TILES):
                cbt = stage.tile([128, R, 2, 8, 128], BF16, name="icbt",
                                 tag="icbt")
                nc.gpsimd.dma_start(
                    cbt[:].rearrange("p r h d f -> p (r h d f)"),
                    wblob_d.ap()[t * ICB_T:(t + 1) * ICB_T]
                    .rearrange("(p f) -> p f", p=128))
                oht = stage.tile([128, R, 2, 8, 64], BF16, name="ioht",
                                 tag="ioht")
                nc.gpsimd.dma_start(
                    oht[:].rearrange("p r h d f -> p (r h d f)"),
                    wblob_d.ap()[IOH_OFF + t * IOH_T:IOH_OFF + (t + 1) * IOH_T]
                    .rearrange("(p f) -> p f", p=128))
                for d in range(8):
                    ps = psT.tile([128, 128], F32, name="pstr", tag="pstr")
                    for h in range(2):
                        for r in range(R):
                            nc.tensor.matmul(ps[:, h * 64:(h + 1) * 64],
                                             cbt[:, r, h, d, :],
                                             oht[:, r, h, d, :],
                                             start=(r == 0), stop=(r == 1))
                    if d % 2:
                        nc.scalar.copy(winT[d][:, t * 128:(t + 1) * 128], ps[:])
                    else:
                        nc.vector.tensor_copy(
                            winT[d][:, t * 128:(t + 1) * 128], ps[:])

        def emit_w2():
            """out_proj dequant: one-hot matmuls, levels+fold accumulated."""
            for ot in range(8):
                cbt = stage.tile([128, R, 2, KK, 128], BF16, name="ocbt",
                                 tag="ocbt")
                nc.gpsimd.dma_start(
                    cbt[:].rearrange("p r f k c -> p (r f k c)"),
                    wblob_d.ap()[OCB_OFF + ot * OCB_T:OCB_OFF + (ot + 1) * OCB_T]
                    .rearrange("(p f) -> p f", p=128))
                oht = stage.tile([128, R, 2, KK, 128], BF16, name="ooht",
                                 tag="ooht")
                nc.gpsimd.dma_start(
                    oht[:].rearrange("p r f k c -> p (r f k c)"),
                    wblob_d.ap()[OOH_OFF + ot * OCB_T:OOH_OFF + (ot + 1) * OCB_T]
                    .rearrange("(p f) -> p f", p=128))
                for kk in range(KK):
                    ps = psT.tile([128, 128], F32, name="pstr", tag="pstr")
                    i = 0
                    for r in range(R):
                        for fh in range(2):
                            nc.tensor.matmul(ps[:], cbt[:, r, fh, kk, :],
                                             oht[:, r, fh, kk, :],
                                             start=(i == 0), stop=(i == 3))
                            i += 1
                    if kk % 2:
                        nc.scalar.copy(wfoldT[kk][:, ot * 128:(ot + 1) * 128],
                                       ps[:])
                    else:
                        nc.vector.tensor_copy(
                            wfoldT[kk][:, ot * 128:(ot + 1) * 128], ps[:])

        def emit_inproj(xnT, hT):
            for f in range(F_TILES):
                pss = [psA.tile([128, 512], F32, name="mm", tag="mm")
                       for _ in range(2)]
                for d in range(8):
                    for qc in range(2):
                        nc.tensor.matmul(
                            pss[qc][:], winT[d][:, f * 128:(f + 1) * 128],
                            xnT[d][:, qc * 512:(qc + 1) * 512],
                            start=(d == 0), stop=(d == 7))
                nc.scalar.copy(hT[f][:, 0:512], pss[0][:])
                nc.vector.tensor_copy(hT[f][:, 512:1024], pss[1][:])

        def emit_attn(hT, xaT):
            vaugs = []
            for hh in range(HEADS_PER_CORE):
                lo, hi = hh * 64, (hh + 1) * 64
                vaug = wpool.tile([128, 8, 65], BF16, name=f"vaug{hh}",
                                  tag=f"vaug{hh}")
                for kvt in range(8):
                    psv = psT.tile([128, 128], BF16, name="pstb", tag="pstr")
                    nc.tensor.matmul(psv[:, :64],
                                     hT[6][lo:hi, kvt * 128:(kvt + 1) * 128],
                                     ident_b[lo:hi, lo:hi], is_transpose=True,
                                     start=True, stop=True)
                    nc.vector.tensor_copy(vaug[:, kvt, 0:64], psv[:, :64])
                nc.vector.memset(vaug[:, :, 64:65], 1.0)
                vaugs.append(vaug)
            for qc in range(2):
                psavs = [psV.tile([65, 512], F32, name=f"av{hh}",
                                  tag=f"av{hh}") for hh in range(2)]
                prev = None
                for kvt in range(8):
                    cur = []
                    for hh in range(HEADS_PER_CORE):
                        lo, hi = hh * 64, (hh + 1) * 64
                        ps = psA.tile([128, 512], F32, name="mm", tag="mm")
                        nc.tensor.matmul(
                            ps[:], hT[5][lo:hi, kvt * 128:(kvt + 1) * 128],
                            hT[4][lo:hi, qc * 512:(qc + 1) * 512],
                            start=True, stop=True)
                        ptk = ptpool.tile([128, 512], BF16, name=f"PT{hh}",
                                          tag=f"PT{hh}")
                        nc.scalar.activation(ptk[:], ps[:], AF.Exp,
                                             scale=SCALE)
                        cur.append(ptk)
                    if prev is not None:
                        for hh in range(HEADS_PER_CORE):
                            nc.tensor.matmul(psavs[hh][:],
                                             vaugs[hh][:, kvt - 1, :],
                                             prev[hh][:],
                                             start=(kvt == 1), stop=False)
                    prev = cur
                for hh in range(HEADS_PER_CORE):
                    nc.tensor.matmul(psavs[hh][:], vaugs[hh][:, 7, :],
                                     prev[hh][:], start=False, stop=True)
                for hh in range(HEADS_PER_CORE):
                    lo, hi = hh * 64, (hh + 1) * 64
                    psav = psavs[hh]
                    rec = small.tile([1, 512], F32R, name="rec", tag="rec")
                    nc.vector.reciprocal(rec[:], psav[64:65, :])
                    psb = psA.tile([64, 512], F32, name="mm", tag="mm")
                    nc.tensor.matmul(psb[:], ones_r[:], rec[:],
                                     start=True, stop=True)
                    bc = small.tile([64, 512], F32, name="bc_sb", tag="bc_sb")
                    nc.vector.tensor_copy(bc[:], psb[:])
                    nc.vector.tensor_tensor(
                        xaT[lo:hi, qc * 512:(qc + 1) * 512],
                        psav[0:64, :], bc[:], op=ALU.mult)

        def emit_outproj(b, hT, xaT):
            for ot in range(8):
                pss = [psA.tile([128, 512], F32, name="mm", tag="mm")
                       for _ in range(2)]
                for kk in range(KK):
                    rhs = hT[kk] if kk < 4 else xaT
                    for qc in range(2):
                        nc.tensor.matmul(
                            pss[qc][:], wfoldT[kk][:, ot * 128:(ot + 1) * 128],
                            rhs[:, qc * 512:(qc + 1) * 512],
                            start=(kk == 0), stop=(kk == KK - 1))
                osb = work.tile([128, 1024], F32, name="osb", tag="osb")
                nc.scalar.copy(osb[:, 0:512], pss[0][:])
                nc.vector.tensor_copy(osb[:, 512:1024], pss[1][:])
                nc.sync.dma_start(
                    o_d.ap()[ot * 128:(ot + 1) * 128, b * N: (b + 1) * N],
                    osb[:])

        # ---------------- emission order --------------------------------
        def alloc_xnT(par):
            return [wpool.tile([128, N], BF16, name=f"xnT{d}",
                               tag=f"xnT{d}p{par}") for d in range(8)]

        # BASS_REPEAT>1 re-emits the body k times for wall-delta timing
        for _rep in range(int(os.environ.get("BASS_REPEAT", "1"))):
            xnT_b = alloc_xnT(0)
            emit_ln(0, xnT_b)
            emit_w1()
            for b in range(B):
                par = b % 2
                hT_b = [wpool.tile([128, N], BF16, name=f"hT{f}",
                                   tag=f"hT{f}p{par}") for f in range(F_TILES)]
                emit_inproj(xnT_b, hT_b)
                if b == 0:
                    emit_w2()
                if b + 1 < B:
                    xnT_next = alloc_xnT(1 - par)
                    emit_ln(b + 1, xnT_next)
                xaT_b = wpool.tile([128, N], BF16, name="xaT", tag=f"xaTp{par}")
                emit_attn(hT_b, xaT_b)
                emit_outproj(b, hT_b, xaT_b)
                if b + 1 < B:
                    xnT_b = xnT_next

    nc.compile()
    _BUILD_CACHE["nc"] = nc
    return nc


def make_in_maps(x, in_codebooks, in_indices, out_codebooks, out_indices):
    """Host-side input marshalling: per-core one-hot index matrices and
    block-diagonal codebook tiles (bf16), plus the flattened activations.

    Pure layout/encoding transforms — all arithmetic (dequant sums, GEMMs,
    LN, SDPA) runs on device.
    """
    x4096 = np.ascontiguousarray(np.asarray(x).reshape(TOK, DIM)
                                 .astype(np.float32))
    in_cb = np.asarray(in_codebooks, np.float32)
    in_idx = np.asarray(in_indices)
    out_cb = np.asarray(out_codebooks, np.float32)
    out_idx = np.asarray(out_indices)
    eye = np.arange(K)

    in_maps = []
    for c in range(NCORES):
        # ---- in_proj: icb (7,128,R,2,8,128), ioh (7,128,R,2,8,64) ----
        rows = np.stack([np.arange(_row_base(c, t), _row_base(c, t) + 128)
                         for t in range(F_TILES)])            # (7,128)
        cl0 = np.array([_row_base(c, t) // 64 for t in range(F_TILES)])

        ivc = in_idx[:, rows, :]                              # (R,7,128,16)
        oh = (ivc[..., None] == eye).astype(NPBF16)           # (R,7,128,16,64)
        # axes: (r, t, (h,m), (d,ci), k) -> (t, ci, k, r, h, d, m)
        oh = oh.reshape(R, F_TILES, 2, 64, 8, 2, K)
        ioh = np.ascontiguousarray(
            oh.transpose(1, 5, 6, 0, 2, 4, 3)                 # t,ci,k,r,h,d,m
            .reshape(F_TILES, 128, R, 2, 8, 64))

        # cb tiles: (r, t, h, d, ci, k, s) from clusters cl0[t]+h
        cl_ids = cl0[:, None] + np.array([0, 1])              # (7,2)
        cbs = in_cb[:, cl_ids]                                # (R,7,2,16,64,64)
        cbs = cbs.reshape(R, F_TILES, 2, 8, 2, K, SUB_IN)
        icb = np.zeros((F_TILES, 128, R, 2, 8, 128), np.float32)
        for ci in range(2):
            # (r,t,h,d,k,s) -> (t,k,r,h,d,s)
            blk = cbs[:, :, :, :, ci].transpose(1, 4, 0, 2, 3, 5)
            icb[:, ci * 64:(ci + 1) * 64, :, :, :,
                ci * 64:(ci + 1) * 64] = blk
        icb = icb.astype(NPBF16)

        # ---- out_proj: ocb/ooh (8,128,R,2,5,128) -------------------------
        cols = _chunk_cols(c)
        gcbk = np.array([g // SUB_OUT for g in cols])          # (10,)
        gsub = np.array([(g % SUB_OUT) // 64 for g in cols])   # (10,)

        # per-chunk codebook slices: (R, 16 clusters, 10, K, 64)
        sel = np.empty((R, OUT_CLUSTERS, 10, K, 64), np.float32)
        for i in range(10):
            sel[:, :, i] = out_cb[:, :, gcbk[i], :,
                                  64 * gsub[i]: 64 * gsub[i] + 64]
        # (r, (fh,ot) cluster, (kk,ci), k, s) -> (ot, ci, k, r, fh, kk, s)
        sel = sel.reshape(R, 2, 8, KK, 2, K, 64)
        ocb = np.zeros((8, 128, R, 2, KK, 128), np.float32)
        for ci in range(2):
            # (r,fh,ot,kk,k,s) -> (ot,k,r,fh,kk,s)
            blk = sel[:, :, :, :, ci].transpose(2, 4, 0, 1, 3, 5)
            ocb[:, ci * 64:(ci + 1) * 64, :, :, :,
                ci * 64:(ci + 1) * 64] = blk
        ocb = ocb.astype(NPBF16)

        ov = out_idx[:, :, gcbk]                               # (R,2048,10)
        ooh_raw = (ov[..., None] == eye).astype(NPBF16)        # (R,2048,10,64)
        # rows: (r, (fh,ot,m), (kk,ci), k) -> (ot, ci, k, r, fh, kk, m)
        ooh_raw = ooh_raw.reshape(R, 2, 8, 128, KK, 2, K)
        ooh = np.ascontiguousarray(
            ooh_raw.transpose(2, 5, 6, 0, 1, 4, 3)
            .reshape(8, 128, R, 2, KK, 128))

        wblob = np.concatenate([
            np.ascontiguousarray(icb).ravel(), ioh.ravel(),
            np.ascontiguousarray(ocb).ravel(), ooh.ravel()])
        assert wblob.shape[0] == WB_TOTAL and wblob.dtype == NPBF16
        in_maps.append({"x4096": x4096, "wblob": wblob})
    return in_maps


def combine_outputs(x, results):
    o_sum = np.zeros((DIM, TOK), np.float64)
    for rmap in results:
        o_sum += rmap["o_t"].astype(np.float64)
    out = np.asarray(x).reshape(TOK, DIM).astype(np.float64) + o_sum.T
    return out.reshape(B, N, DIM).astype(np.float32)


def kernel(x, in_codebooks, in_indices, out_codebooks, out_indices):
    nc = _build_nc()
    in_maps = make_in_maps(x, in_codebooks, in_indices,
                           out_codebooks, out_indices)
    res = run_bass_kernel_spmd(nc, in_maps, core_ids=list(range(NCORES)))
    return combine_outputs(x, [res.results[c] for c in range(NCORES)])


# revision 9
# speedup vs baseline: 2.4835x; 1.2147x over previous
"""Trainium2 Bass kernel for nn_HKRPQParallelBlock (RPQ-quantized parallel
transformer block: LN -> in_proj (dequant GEMM) -> [MLP | SDPA] -> out_proj
(dequant GEMM) -> fold + residual).

Sharding (8 cores, zero device-to-device communication):
  - Each core computes a 896-row slice of in_proj (512 MLP rows + q/k/v rows of
    2 heads), cluster-aligned so the RPQ structure stays uniform.
  - Each core runs SDPA for its 2 heads over all 4 batches.
  - out_proj is sharded over its contraction dim (the 640 activation features
    this core produced); every core emits a full partial (1024, 4096) output
    and the host sums the 8 partials (+ residual).
  - The OUT_OUT->DIM fold (o[:, :1024] + o[:, 1024:]) is folded into the
    dequantized weight before the GEMM, halving out_proj FLOPs.

v2 vs v1:
  - RPQ dequant runs as one-hot x codebook matmuls on the PE (bf16) instead
    of SWDGE DMA gathers: the host ships pre-built one-hot index matrices and
    block-diagonal codebook tiles; the PE contracts them straight into the
    transposed weight layout the GEMMs need. No gather descriptors, no Pool
    engine DGE work, and the dequant output needs no separate transpose pass.
  - All matmul operands are bf16 (PSUM stays f32), halving SBUF footprint,
    DMA bytes and vector/scalar element work. PE transposes run 1 cycle/row.
  - DMAs are fewer and bigger, split across the sync (x in / o out) and
    vector (weights) queues; PSUM->SBUF copies are spread over the scalar,
    vector and pool engines.
"""

import os
import numpy as np
import concourse.bass as bass
import concourse.bacc as bacc
import concourse.tile as tile
import concourse.mybir as mybir
from concourse.bass_utils import run_bass_kernel_spmd
from concourse.masks import make_identity
from contextlib import ExitStack

F32 = mybir.dt.float32
F32R = mybir.dt.float32r
BF16 = mybir.dt.bfloat16
AF = mybir.ActivationFunctionType
ALU = mybir.AluOpType

NCORES = 8
DIM = 1024
HEADS = 16
HD = 64
MLP = 4 * DIM                 # 4096
IN_OUT = MLP + 3 * DIM        # 7168
OUT_IN = MLP + DIM            # 5120
OUT_OUT = 2 * DIM             # 2048
R = 2
K = 64
NCB = 16
SUB_IN = 64
SUB_OUT = 320
IN_CLUSTERS = 112
OUT_CLUSTERS = 16
B, N = 4, 1024
TOK = B * N                   # 4096
EPS = 1e-5
SCALE = HD ** -0.5            # 0.125

F_TILES = 7                   # per-core in_proj feature tiles of 128 rows
MLP_PER_CORE = MLP // NCORES  # 512
HEADS_PER_CORE = 2
KK = 5                        # out_proj contraction tiles of 128 per core

NPBF16 = mybir.dt.np(BF16)

# packed weight-blob section sizes (bf16 elements)
ICB_T = 128 * R * 2 * 8 * 128          # per f-tile: 524288
IOH_T = 128 * R * 2 * 8 * 64           # per f-tile: 262144
OCB_T = 128 * R * 2 * KK * 128         # per out-tile: 327680
IOH_OFF = F_TILES * ICB_T              # 3670016
OCB_OFF = IOH_OFF + F_TILES * IOH_T    # 5505024
OOH_OFF = OCB_OFF + 8 * OCB_T          # 8126464
WB_TOTAL = OOH_OFF + 8 * OCB_T         # 10747904

_BUILD_CACHE = {}


def _row_base(core, t):
    """Global in_proj row of the first row of per-core feature tile t."""
    if t < 4:
        return MLP_PER_CORE * core + 128 * t
    return MLP + DIM * (t - 4) + 128 * core


def _chunk_cols(core):
    """The 10 global out_proj contraction columns (as 64-wide chunks) this
    core owns, in rhs order: 8 MLP chunks then 2 attention chunks."""
    return [MLP_PER_CORE * core + 64 * k for k in range(8)] + \
           [MLP + 128 * core + 64 * k for k in range(2)]


def _build_nc():
    if "nc" in _BUILD_CACHE:
        return _BUILD_CACHE["nc"]

    nc = bacc.Bacc("TRN2", target_bir_lowering=False, debug=False,
                   num_devices=NCORES)

    x_d = nc.dram_tensor("x4096", (TOK, DIM), BF16, kind="ExternalInput")
    # One packed bf16 blob for all dequant operands (fewer per-call buffer
    # args + fewer, bigger DMAs): [icb | ioh | ocb | ooh], each section
    # t/ot-major with a contiguous [128, free] tile per slice.
    wblob_d = nc.dram_tensor("wblob", (WB_TOTAL,), BF16, kind="ExternalInput")
    o_d = nc.dram_tensor("o_t", (DIM, TOK), F32, kind="ExternalOutput")

    with ExitStack() as ctx, nc.allow_low_precision(reason="bf16 matmul feeds"):
        tc = ctx.enter_context(tile.TileContext(nc))
        const = ctx.enter_context(tc.tile_pool(name="const", bufs=1))
        wpool = ctx.enter_context(tc.tile_pool(name="wpool", bufs=1))
        stage = ctx.enter_context(tc.tile_pool(name="stage", bufs=2))
        work = ctx.enter_context(tc.tile_pool(name="work", bufs=2))
        small = ctx.enter_context(tc.tile_pool(name="small", bufs=2))
        psA = ctx.enter_context(tc.tile_pool(name="psA", bufs=4, space="PSUM"))
        psT = ctx.enter_context(tc.tile_pool(name="psT", bufs=2, space="PSUM"))
        psV = ctx.enter_context(tc.tile_pool(name="psV", bufs=1, space="PSUM"))
        ptpool = ctx.enter_context(tc.tile_pool(name="ptpool", bufs=3))

        ident_f = const.tile([128, 128], F32, tag="ident_f")
        make_identity(nc, ident_f[:])
        ident_b = const.tile([128, 128], BF16, tag="ident_b")
        nc.vector.tensor_copy(ident_b[:], ident_f[:])
        ones_f = const.tile([128, 1], F32, tag="ones_f")
        nc.gpsimd.memset(ones_f[:], 1.0)
        ones_r = const.tile([1, 64], F32R, tag="ones_r")
        nc.vector.tensor_copy(ones_r[:], ones_f[:1, :].to_broadcast([1, 64]))

        winT = [wpool.tile([128, F_TILES * 128], BF16, name=f"winT{d}",
                           tag=f"winT{d}") for d in range(8)]
        wfoldT = [wpool.tile([128, 1024], BF16, name=f"wfT{kk}",
                             tag=f"wfT{kk}") for kk in range(KK)]

        # ---------------- emission helpers ------------------------------
        def emit_ln(b, xnT):
            """LayerNorm + PE transpose of batch b tokens into xnT tiles."""
            for tt in range(8):
                xt = stage.tile([128, DIM], BF16, name="xt", tag="xt")
                nc.sync.dma_start(
                    xt[:], x_d.ap()[b * N + tt * 128: b * N + (tt + 1) * 128, :])
                bstat = small.tile([128, 2, 6], F32, name="bstat", tag="bstat")
                nc.vector.bn_stats(bstat[:, 0, :], xt[:, :512])
                nc.vector.bn_stats(bstat[:, 1, :], xt[:, 512:])
                baggr = small.tile([128, 2], F32, name="baggr", tag="baggr")
                nc.vector.bn_aggr(baggr[:], bstat[:])
                veps = small.tile([128, 1], F32, name="veps", tag="veps")
                nc.vector.tensor_scalar_add(veps[:], baggr[:, 1:2], EPS)
                sd = small.tile([128, 1], F32, name="sd", tag="sd")
                nc.scalar.sqrt(sd[:], veps[:])
                rs = small.tile([128, 1], F32, name="rs", tag="rs")
                nc.vector.reciprocal(rs[:], sd[:])
                xn = stage.tile([128, DIM], BF16, name="xn", tag="xn")
                nc.vector.tensor_scalar(xn[:], xt[:], baggr[:, 0:1], rs[:],
                                        op0=ALU.subtract, op1=ALU.mult)
                for d in range(8):
                    pst = psT.tile([128, 128], BF16, name="pstb", tag="pstr")
                    nc.tensor.matmul(pst[:], xn[:, d * 128:(d + 1) * 128],
                                     ident_b[:], is_transpose=True,
                                     start=True, stop=True)
                    if d % 2:
                        nc.scalar.copy(
                            xnT[d][:, tt * 128:(tt + 1) * 128], pst[:])
                    else:
                        nc.vector.tensor_copy(
                            xnT[d][:, tt * 128:(tt + 1) * 128], pst[:])

        def emit_w1():
            """in_proj dequant: one-hot x codebook matmuls, 2 levels fused."""
            for t in range(F_TILES):
                cbt = stage.tile([128, R, 2, 8, 128], BF16, name="icbt",
                                 tag="icbt")
                nc.gpsimd.dma_start(
                    cbt[:].rearrange("p r h d f -> p (r h d f)"),
                    wblob_d.ap()[t * ICB_T:(t + 1) * ICB_T]
                    .rearrange("(p f) -> p f", p=128))
                oht = stage.tile([128, R, 2, 8, 64], BF16, name="ioht",
                                 tag="ioht")
                nc.gpsimd.dma_start(
                    oht[:].rearrange("p r h d f -> p (r h d f)"),
                    wblob_d.ap()[IOH_OFF + t * IOH_T:IOH_OFF + (t + 1) * IOH_T]
                    .rearrange("(p f) -> p f", p=128))
                for d in range(8):
                    ps = psT.tile([128, 128], F32, name="pstr", tag="pstr")
                    for h in range(2):
                        for r in range(R):
                            nc.tensor.matmul(ps[:, h * 64:(h + 1) * 64],
                                             cbt[:, r, h, d, :],
                                             oht[:, r, h, d, :],
                                             start=(r == 0), stop=(r == 1))
                    if d % 2:
                        nc.scalar.copy(winT[d][:, t * 128:(t + 1) * 128], ps[:])
                    else:
                        nc.vector.tensor_copy(
                            winT[d][:, t * 128:(t + 1) * 128], ps[:])

        def emit_w2():
            """out_proj dequant: one-hot matmuls, levels+fold accumulated."""
            for ot in range(8):
                cbt = stage.tile([128, R, 2, KK, 128], BF16, name="ocbt",
                                 tag="ocbt")
                nc.gpsimd.dma_start(
                    cbt[:].rearrange("p r f k c -> p (r f k c)"),
                    wblob_d.ap()[OCB_OFF + ot * OCB_T:OCB_OFF + (ot + 1) * OCB_T]
                    .rearrange("(p f) -> p f", p=128))
                oht = stage.tile([128, R, 2, KK, 128], BF16, name="ooht",
                                 tag="ooht")
                nc.gpsimd.dma_start(
                    oht[:].rearrange("p r f k c -> p (r f k c)"),
                    wblob_d.ap()[OOH_OFF + ot * OCB_T:OOH_OFF + (ot + 1) * OCB_T]
                    .rearrange("(p f) -> p f", p=128))
                for kk in range(KK):
                    ps = psT.tile([128, 128], F32, name="pstr", tag="pstr")
                    i = 0
                    for r in range(R):
                        for fh in range(2):
                            nc.tensor.matmul(ps[:], cbt[:, r, fh, kk, :],
                                             oht[:, r, fh, kk, :],
                                             start=(i == 0), stop=(i == 3))
                            i += 1
                    if kk % 2:
                        nc.scalar.copy(wfoldT[kk][:, ot * 128:(ot + 1) * 128],
                                       ps[:])
                    else:
                        nc.vector.tensor_copy(
                            wfoldT[kk][:, ot * 128:(ot + 1) * 128], ps[:])

        def emit_inproj(xnT, hT):
            for f in range(F_TILES):
                pss = [psA.tile([128, 512], F32, name="mm", tag="mm")
                       for _ in range(2)]
                for d in range(8):
                    for qc in range(2):
                        nc.tensor.matmul(
                            pss[qc][:], winT[d][:, f * 128:(f + 1) * 128],
                            xnT[d][:, qc * 512:(qc + 1) * 512],
                            start=(d == 0), stop=(d == 7))
                nc.scalar.copy(hT[f][:, 0:512], pss[0][:])
                nc.vector.tensor_copy(hT[f][:, 512:1024], pss[1][:])

        def emit_attn(hT, xaT):
            vaugs = []
            for hh in range(HEADS_PER_CORE):
                lo, hi = hh * 64, (hh + 1) * 64
                vaug = wpool.tile([128, 8, 65], BF16, name=f"vaug{hh}",
                                  tag=f"vaug{hh}")
                for kvt in range(8):
                    psv = psT.tile([128, 128], BF16, name="pstb", tag="pstr")
                    nc.tensor.matmul(psv[:, :64],
                                     hT[6][lo:hi, kvt * 128:(kvt + 1) * 128],
                                     ident_b[lo:hi, lo:hi], is_transpose=True,
                                     start=True, stop=True)
                    nc.vector.tensor_copy(vaug[:, kvt, 0:64], psv[:, :64])
                nc.vector.memset(vaug[:, :, 64:65], 1.0)
                vaugs.append(vaug)
            for qc in range(2):
                psavs = [psV.tile([65, 512], F32, name=f"av{hh}",
                                  tag=f"av{hh}") for hh in range(2)]
                prev = None
                for kvt in range(8):
                    cur = []
                    for hh in range(HEADS_PER_CORE):
                        lo, hi = hh * 64, (hh + 1) * 64
                        ps = psA.tile([128, 512], F32, name="mm", tag="mm")
                        nc.tensor.matmul(
                            ps[:], hT[5][lo:hi, kvt * 128:(kvt + 1) * 128],
                            hT[4][lo:hi, qc * 512:(qc + 1) * 512],
                            start=True, stop=True)
                        ptk = ptpool.tile([128, 512], BF16, name=f"PT{hh}",
                                          tag=f"PT{hh}")
                        nc.scalar.activation(ptk[:], ps[:], AF.Exp,
                                             scale=SCALE)
                        cur.append(ptk)
                    if prev is not None:
                        for hh in range(HEADS_PER_CORE):
                            nc.tensor.matmul(psavs[hh][:],
                                             vaugs[hh][:, kvt - 1, :],
                                             prev[hh][:],
                                             start=(kvt == 1), stop=False)
                    prev = cur
                for hh in range(HEADS_PER_CORE):
                    nc.tensor.matmul(psavs[hh][:], vaugs[hh][:, 7, :],
                                     prev[hh][:], start=False, stop=True)
                for hh in range(HEADS_PER_CORE):
                    lo, hi = hh * 64, (hh + 1) * 64
                    psav = psavs[hh]
                    rec = small.tile([1, 512], F32R, name="rec", tag="rec")
                    nc.vector.reciprocal(rec[:], psav[64:65, :])
                    psb = psA.tile([64, 512], F32, name="mm", tag="mm")
                    nc.tensor.matmul(psb[:], ones_r[:], rec[:],
                                     start=True, stop=True)
                    bc = small.tile([64, 512], F32, name="bc_sb", tag="bc_sb")
                    nc.vector.tensor_copy(bc[:], psb[:])
                    nc.vector.tensor_tensor(
                        xaT[lo:hi, qc * 512:(qc + 1) * 512],
                        psav[0:64, :], bc[:], op=ALU.mult)

        def emit_outproj(b, hT, xaT):
            for ot in range(8):
                pss = [psA.tile([128, 512], F32, name="mm", tag="mm")
                       for _ in range(2)]
                for kk in range(KK):
                    rhs = hT[kk] if kk < 4 else xaT
                    for qc in range(2):
                        nc.tensor.matmul(
                            pss[qc][:], wfoldT[kk][:, ot * 128:(ot + 1) * 128],
                            rhs[:, qc * 512:(qc + 1) * 512],
                            start=(kk == 0), stop=(kk == KK - 1))
                osb = work.tile([128, 1024], F32, name="osb", tag="osb")
                nc.scalar.copy(osb[:, 0:512], pss[0][:])
                nc.vector.tensor_copy(osb[:, 512:1024], pss[1][:])
                nc.sync.dma_start(
                    o_d.ap()[ot * 128:(ot + 1) * 128, b * N: (b + 1) * N],
                    osb[:])

        # ---------------- emission order --------------------------------
        def alloc_xnT(par):
            return [wpool.tile([128, N], BF16, name=f"xnT{d}",
                               tag=f"xnT{d}p{par}") for d in range(8)]

        # BASS_REPEAT>1 re-emits the body k times for wall-delta timing
        for _rep in range(int(os.environ.get("BASS_REPEAT", "1"))):
            xnT_b = alloc_xnT(0)
            emit_ln(0, xnT_b)
            emit_w1()
            for b in range(B):
                par = b % 2
                hT_b = [wpool.tile([128, N], BF16, name=f"hT{f}",
                                   tag=f"hT{f}p{par}") for f in range(F_TILES)]
                emit_inproj(xnT_b, hT_b)
                if b == 0:
                    emit_w2()
                if b + 1 < B:
                    xnT_next = alloc_xnT(1 - par)
                    emit_ln(b + 1, xnT_next)
                xaT_b = wpool.tile([128, N], BF16, name="xaT", tag=f"xaTp{par}")
                emit_attn(hT_b, xaT_b)
                emit_outproj(b, hT_b, xaT_b)
                if b + 1 < B:
                    xnT_b = xnT_next

    nc.compile()
    _BUILD_CACHE["nc"] = nc
    return nc


def make_in_maps(x, in_codebooks, in_indices, out_codebooks, out_indices):
    """Host-side input marshalling: per-core one-hot index matrices and
    block-diagonal codebook tiles (bf16), plus the flattened activations.

    Pure layout/encoding transforms — all arithmetic (dequant sums, GEMMs,
    LN, SDPA) runs on device.
    """
    x4096 = np.ascontiguousarray(np.asarray(x).reshape(TOK, DIM)
                                 .astype(NPBF16))
    in_cb = np.asarray(in_codebooks, np.float32)
    in_idx = np.asarray(in_indices)
    out_cb = np.asarray(out_codebooks, np.float32)
    out_idx = np.asarray(out_indices)
    eye = np.arange(K)

    in_maps = []
    for c in range(NCORES):
        # ---- in_proj: icb (7,128,R,2,8,128), ioh (7,128,R,2,8,64) ----
        rows = np.stack([np.arange(_row_base(c, t), _row_base(c, t) + 128)
                         for t in range(F_TILES)])            # (7,128)
        cl0 = np.array([_row_base(c, t) // 64 for t in range(F_TILES)])

        ivc = in_idx[:, rows, :]                              # (R,7,128,16)
        oh = (ivc[..., None] == eye).astype(NPBF16)           # (R,7,128,16,64)
        # axes: (r, t, (h,m), (d,ci), k) -> (t, ci, k, r, h, d, m)
        oh = oh.reshape(R, F_TILES, 2, 64, 8, 2, K)
        ioh = np.ascontiguousarray(
            oh.transpose(1, 5, 6, 0, 2, 4, 3)                 # t,ci,k,r,h,d,m
            .reshape(F_TILES, 128, R, 2, 8, 64))

        # cb tiles: (r, t, h, d, ci, k, s) from clusters cl0[t]+h
        cl_ids = cl0[:, None] + np.array([0, 1])              # (7,2)
        cbs = in_cb[:, cl_ids]                                # (R,7,2,16,64,64)
        cbs = cbs.reshape(R, F_TILES, 2, 8, 2, K, SUB_IN)
        icb = np.zeros((F_TILES, 128, R, 2, 8, 128), np.float32)
        for ci in range(2):
            # (r,t,h,d,k,s) -> (t,k,r,h,d,s)
            blk = cbs[:, :, :, :, ci].transpose(1, 4, 0, 2, 3, 5)
            icb[:, ci * 64:(ci + 1) * 64, :, :, :,
                ci * 64:(ci + 1) * 64] = blk
        icb = icb.astype(NPBF16)

        # ---- out_proj: ocb/ooh (8,128,R,2,5,128) -------------------------
        cols = _chunk_cols(c)
        gcbk = np.array([g // SUB_OUT for g in cols])          # (10,)
        gsub = np.array([(g % SUB_OUT) // 64 for g in cols])   # (10,)

        # per-chunk codebook slices: (R, 16 clusters, 10, K, 64)
        sel = np.empty((R, OUT_CLUSTERS, 10, K, 64), np.float32)
        for i in range(10):
            sel[:, :, i] = out_cb[:, :, gcbk[i], :,
                                  64 * gsub[i]: 64 * gsub[i] + 64]
        # (r, (fh,ot) cluster, (kk,ci), k, s) -> (ot, ci, k, r, fh, kk, s)
        sel = sel.reshape(R, 2, 8, KK, 2, K, 64)
        ocb = np.zeros((8, 128, R, 2, KK, 128), np.float32)
        for ci in range(2):
            # (r,fh,ot,kk,k,s) -> (ot,k,r,fh,kk,s)
            blk = sel[:, :, :, :, ci].transpose(2, 4, 0, 1, 3, 5)
            ocb[:, ci * 64:(ci + 1) * 64, :, :, :,
                ci * 64:(ci + 1) * 64] = blk
        ocb = ocb.astype(NPBF16)

        ov = out_idx[:, :, gcbk]                               # (R,2048,10)
        ooh_raw = (ov[..., None] == eye).astype(NPBF16)        # (R,2048,10,64)
        # rows: (r, (fh,ot,m), (kk,ci), k) -> (ot, ci, k, r, fh, kk, m)
        ooh_raw = ooh_raw.reshape(R, 2, 8, 128, KK, 2, K)
        ooh = np.ascontiguousarray(
            ooh_raw.transpose(2, 5, 6, 0, 1, 4, 3)
            .reshape(8, 128, R, 2, KK, 128))

        wblob = np.concatenate([
            np.ascontiguousarray(icb).ravel(), ioh.ravel(),
            np.ascontiguousarray(ocb).ravel(), ooh.ravel()])
        assert wblob.shape[0] == WB_TOTAL and wblob.dtype == NPBF16
        in_maps.append({"x4096": x4096, "wblob": wblob})
    return in_maps


def combine_outputs(x, results):
    o_sum = np.zeros((DIM, TOK), np.float64)
    for rmap in results:
        o_sum += rmap["o_t"].astype(np.float64)
    out = np.asarray(x).reshape(TOK, DIM).astype(np.float64) + o_sum.T
    return out.reshape(B, N, DIM).astype(np.float32)


def kernel(x, in_codebooks, in_indices, out_codebooks, out_indices):
    nc = _build_nc()
    in_maps = make_in_maps(x, in_codebooks, in_indices,
                           out_codebooks, out_indices)
    res = run_bass_kernel_spmd(nc, in_maps, core_ids=list(range(NCORES)))
    return combine_outputs(x, [res.results[c] for c in range(NCORES)])
